# revision 51
# baseline (speedup 1.0000x reference)
"""Causal attention (B=4, S=4096, D=64, fp32) on 8 Trainium2 NeuronCores.

Sharding: two SPMD programs dispatched concurrently on disjoint device
sets. Within a batch, the 4096 q rows form 8 chunks of 512 columns;
chunk c needs k-tiles 0..4c+3 (causal). Program A (cores 0-3, one batch
each) takes chunks {2,0,5,7} (k-tile counts {12,4,24,32}); program B
(cores 4-7) takes chunks {1,3,4,6} (counts {8,16,20,28}). Both sum to
72 tile-passes per core - perfectly balanced. A PE warmup block keeps
the HAM clock-gate from running the stream at 1.2 GHz.

Layout: scores are computed transposed, S^T[k,q] = K Q^T, with the
contraction dim d on SBUF partitions, so softmax normalization can be
deferred (a ones-column appended to V accumulates the row sums during
the P^T V matmul) and P^T feeds the PV matmul with no transposes. The
last up-to-8 k-tiles of each chunk come from per-chunk "slab" inputs;
the final 4 are the diagonal tiles, whose triangles are zeroed by a
DVE multiply with a host-supplied triangular mask at fixed program
positions. All matmul operands are fp16 (PE runs 1 cycle/row);
accumulation stays fp32 in PSUM.
"""

import numpy as np

import jax
import concourse.bass as bass  # noqa: F401
import concourse.mybir as mybir
from concourse import bacc
from concourse import bass2jax
from concourse.tile import TileContext

B, S, D = 4, 4096, 64
NCORES = 8
SLOT_A = (12, 4, 24, 32)  # program A: chunks {2,0,5,7} of a batch (72 tiles)
SLOT_B = (8, 16, 20, 28)  # program B: chunks {1,3,4,6} (72 tiles)
F32 = mybir.dt.float32
F16 = mybir.dt.float16

_cache = {}


def _chunk_index(slot_c, m):
    # chunk whose causal need equals slot_c[m]
    return slot_c[m] // 4 - 1


def _build_program(slot_c, warmup_n):
    n_shared = [max(c - 8, 0) for c in slot_c]
    n_slab = [min(c, 8) for c in slot_c]
    max_shared = max(n_shared)

    nc = bacc.Bacc("TRN2", target_bir_lowering=False, debug=False)
    qt_d = nc.declare_dram_parameter("qt", [65, 2048], F16, isOutput=False)
    ktm_d = nc.declare_dram_parameter(
        "ktm", [64, 128 * max_shared], F16, isOutput=False
    )
    kts_d = nc.declare_dram_parameter("kts", [65, 4096], F16, isOutput=False)
    vm_d = nc.declare_dram_parameter(
        "vm", [128, 65 * max_shared], F16, isOutput=False
    )
    vs_d = nc.declare_dram_parameter("vs", [128, 2080], F16, isOutput=False)
    id_d = nc.declare_dram_parameter("id65", [65, 65], F32, isOutput=False)
    mk_d = nc.declare_dram_parameter("mask", [128, 512], F16, isOutput=False)
    o_d = nc.declare_dram_parameter("o", [2048, 64], F32, isOutput=True)
    EXP = mybir.ActivationFunctionType.Exp

    with TileContext(nc) as tc:
        with (
            tc.tile_pool(name="cons", bufs=1) as cons,
            tc.tile_pool(name="data", bufs=1) as data,
            tc.tile_pool(name="pp", bufs=4) as pp,
            tc.tile_pool(name="ep", bufs=2) as ep,
            tc.tile_pool(name="ps_sc", bufs=3, space="PSUM") as ps_sc,
            tc.tile_pool(name="ps_acc", bufs=1, space="PSUM") as ps_acc,
            tc.tile_pool(name="ps_t", bufs=1, space="PSUM") as ps_t,
        ):
            warm = cons.tile([128, 512], F16)
            nc.vector.memset(warm[:], 0.0)
            for w in range(warmup_n):
                wp = ps_sc.tile([128, 1024], F32, tag="sc")
                nc.tensor.matmul(
                    wp[:, 0:512], warm[:, 0:128], warm[:], start=True, stop=True
                )

            qt = data.tile([65, 2048], F16)
            kts = data.tile([65, 4096], F16)
            vs = data.tile([128, 2080], F16)
            ktm = data.tile([64, 128 * max_shared], F16)
            vm = data.tile([128, 65 * max_shared], F16)

            def dma_slot(m):
                nc.sync.dma_start(
                    out=kts[:, 1024 * m : 1024 * m + 128 * n_slab[m]],
                    in_=kts_d[:, 1024 * m : 1024 * m + 128 * n_slab[m]],
                )
                nc.sync.dma_start(
                    out=vs[:, 520 * m : 520 * m + 65 * n_slab[m]],
                    in_=vs_d[:, 520 * m : 520 * m + 65 * n_slab[m]],
                )

            def dma_main(lo, hi):  # shared k-tiles [lo, hi)
                if hi <= lo:
                    return
                nc.sync.dma_start(
                    out=ktm[:, 128 * lo : 128 * hi], in_=ktm_d[:, 128 * lo : 128 * hi]
                )
                nc.sync.dma_start(
                    out=vm[:, 65 * lo : 65 * hi], in_=vm_d[:, 65 * lo : 65 * hi]
                )

            # ordered by first use so no chunk ever waits on its k data;
            # after each slot's slabs, prefetch one 6-tile piece toward the
            # largest future shared-tile need
            nc.sync.dma_start(out=qt[:], in_=qt_d[:])
            ident = cons.tile([65, 65], F32)
            mask = cons.tile([128, 512], F16)
            done = 0
            for m in range(4):
                ns = n_shared[m]
                while done < ns:
                    step = min(6, ns - done)
                    dma_main(done, done + step)
                    done += step
                dma_slot(m)
                if m == 0:
                    nc.sync.dma_start(out=mask[:], in_=mk_d[:])
                    nc.sync.dma_start(out=ident[:], in_=id_d[:])
                future = max(n_shared[m:])
                if done < future:
                    step = min(6, future - done)
                    dma_main(done, done + step)
                    done += step

            pending = []  # (emit_fn, pt, gang, after_fn) across chunks

            def pump(limit):
                while len(pending) > limit:
                    fn, pt_, gang_, after = pending.pop(0)
                    fn(pt_, gang_)
                    if after is not None:
                        after()

            for m in range(4):
                C = slot_c[m]
                ns = n_shared[m]
                # last chunk: host orders the slab diag-tiles first so the
                # chunk tail is mask-free (shorter kernel-exit chain)
                diag_first = m == 3 and ns >= 4
                q_sl = slice(512 * m, 512 * (m + 1))
                acc = ps_acc.tile([65, 512], F32, tag="acc")

                def tile_geom(t, C=C, ns=ns, diag_first=diag_first):
                    # diagonal tile g only needs q-cols [128g, 512)
                    g = (t - ns) if diag_first else (t - (C - 4))
                    if 0 <= g <= 3:
                        off = 128 * g
                    else:
                        off = 0
                    return (g if 0 <= g <= 3 else -1), off, 512 - off

                def emit_pv(pt, gang, C=C, m=m, ns=ns, acc=acc, tile_geom=tile_geom):
                    pcol = 0
                    for t in gang:
                        g, off, w = tile_geom(t)
                        ptile = pt[:, pcol : pcol + w]
                        pcol += w
                        if g >= 0:
                            nc.vector.tensor_mul(ptile, ptile, mask[:, :w])
                        if t < ns:
                            vt = vm[:, 65 * t : 65 * (t + 1)]
                        else:
                            p = t - ns
                            vt = vs[:, 520 * m + 65 * p : 520 * m + 65 * (p + 1)]
                        nc.tensor.matmul(
                            acc[:, off:512],
                            vt,
                            ptile,
                            start=(t == 0),
                            stop=(t == C - 1),
                        )

                def make_epilogue(m=m, acc=acc):
                    last = m == 3

                    def epilogue():
                        osb = ep.tile([65, 512], F32, tag="osb")
                        if last:
                            # split the PSUM->SBUF copy across DVE and ACT
                            nc.vector.tensor_copy(osb[:, 0:256], acc[:, 0:256])
                            nc.scalar.activation(
                                osb[:, 256:512],
                                acc[:, 256:512],
                                mybir.ActivationFunctionType.Copy,
                            )
                        else:
                            nc.vector.tensor_copy(osb[:], acc[:])
                        oo = ep.tile([128, 256], F32, tag="oo")
                        tp = ps_t.tile([128, 260], F32, tag="tp")
                        for j in range(4):
                            nc.tensor.transpose(
                                tp[:, 65 * j : 65 * (j + 1)],
                                osb[:, 128 * j : 128 * (j + 1)],
                                ident[:],
                            )
                        recs = []
                        for j in range(4):
                            rec = ep.tile([128, 1], F32, tag="rec", bufs=4)
                            nc.vector.reciprocal(
                                rec[:], tp[:, 65 * j + 64 : 65 * j + 65]
                            )
                            recs.append(rec)
                        for j in range(4):
                            if last and j % 2:
                                nc.scalar.activation(
                                    oo[:, 64 * j : 64 * (j + 1)],
                                    tp[:, 65 * j : 65 * j + 64],
                                    mybir.ActivationFunctionType.Copy,
                                    scale=recs[j][:],
                                )
                            else:
                                nc.vector.tensor_scalar_mul(
                                    oo[:, 64 * j : 64 * (j + 1)],
                                    tp[:, 65 * j : 65 * j + 64],
                                    recs[j][:],
                                )
                        halves = ((0, 2), (2, 4)) if last else ((0, 4),)
                        for j0, j1 in halves:
                            nc.sync.dma_start(
                                out=o_d[
                                    512 * m + 128 * j0 : 512 * m + 128 * j1, :
                                ].rearrange("(j p) d -> p j d", j=j1 - j0),
                                in_=oo[:, 64 * j0 : 64 * j1].rearrange(
                                    "p (j d) -> p j d", j=j1 - j0
                                ),
                            )

                    return epilogue

                n_gangs = (C + 1) // 2
                for gi, t0 in enumerate(range(0, C, 2)):
                    gang = list(range(t0, min(t0 + 2, C)))
                    sc = ps_sc.tile([128, 1024], F32, tag="sc")
                    pcol = 0
                    for t in gang:
                        g, off, w = tile_geom(t)
                        if t < ns:
                            lhsT = ktm[:, 128 * t : 128 * (t + 1)]
                            rhs = qt[0:64, q_sl]
                        else:
                            p = t - ns
                            lhsT = kts[
                                :, 1024 * m + 128 * p : 1024 * m + 128 * (p + 1)
                            ]
                            rhs = qt[0:65, q_sl]
                        if off:
                            rhs = rhs[:, off:512]
                        nc.tensor.matmul(
                            sc[:, pcol : pcol + w], lhsT, rhs, start=True, stop=True
                        )
                        pcol += w
                    pt = pp.tile([128, 1024], F16, tag="pt")
                    nc.scalar.activation(
                        pt[:, :pcol], sc[:, :pcol], EXP, scale=0.125
                    )
                    after = make_epilogue() if gi == n_gangs - 1 else None
                    pending.append((emit_pv, pt, gang, after))
                    pump(2)
            pump(0)

    nc.compile()
    return nc


def _prep_core_inputs(slot_c, b, query, key, value):
    n_shared = [max(c - 8, 0) for c in slot_c]
    n_slab = [min(c, 8) for c in slot_c]
    max_shared = max(n_shared)

    qt = np.zeros((65, 2048), np.float16)
    qt[64, :] = 1.0
    kts = np.zeros((65, 4096), np.float16)
    vs = np.zeros((128, 2080), np.float16)
    ktm = np.ascontiguousarray(key[b, : 128 * max_shared, :].T.astype(np.float16))
    vaug = np.ones((S, 65), np.float16)
    vaug[:, :64] = value[b]
    vm = np.ascontiguousarray(
        vaug[: 128 * max_shared]
        .reshape(max_shared, 128, 65)
        .transpose(1, 0, 2)
        .reshape(128, 65 * max_shared)
    )
    for m in range(4):
        c = _chunk_index(slot_c, m)
        n = slot_c[m]
        diag_first = m == 3 and n_shared[m] >= 4
        qt[:64, 512 * m : 512 * (m + 1)] = query[b, 512 * c : 512 * (c + 1), :].T
        for p in range(n_slab[m]):
            if diag_first:
                t = (n - 4 + p) if p < 4 else (n - 8 + (p - 4))
            else:
                t = n_shared[m] + p
            col = slice(1024 * m + 128 * p, 1024 * m + 128 * (p + 1))
            vcol = slice(520 * m + 65 * p, 520 * m + 65 * (p + 1))
            kts[:64, col] = key[b, 128 * t : 128 * (t + 1), :].T
            vs[:, vcol] = vaug[128 * t : 128 * (t + 1), :]
    id65 = np.eye(65, dtype=np.float32)
    mask = np.triu(np.ones((128, 512), dtype=np.float16))
    return {"qt": qt, "ktm": ktm, "kts": kts, "vm": vm, "vs": vs,
            "id65": id65, "mask": mask}


def _make_runner(nc, devices):
    """Vendored multi-core run_bass_via_pjrt with an explicit device set,
    split into an async dispatch and a blocking unpack."""
    from jax.sharding import Mesh, PartitionSpec

    bass2jax.install_neuronx_cc_hook()
    n = len(devices)
    partition_name = nc.partition_id_tensor.name if nc.partition_id_tensor else None
    in_names, out_names, out_avals, zero_outs = [], [], [], []
    for alloc in nc.m.functions[0].allocations:
        if not isinstance(alloc, mybir.MemoryLocationSet):
            continue
        name = alloc.memorylocations[0].name
        if alloc.kind == "ExternalInput":
            if name != partition_name:
                in_names.append(name)
        elif alloc.kind == "ExternalOutput":
            out_names.append(name)
            shape = tuple(alloc.tensor_shape)
            dtype = mybir.dt.np(alloc.dtype)
            out_avals.append(jax.core.ShapedArray(shape, dtype))
            zero_outs.append(np.zeros(shape, dtype))
    n_params = len(in_names)
    all_in = list(in_names) + list(out_names)
    if partition_name is not None:
        all_in.append(partition_name)
    all_in = tuple(all_in)
    donate = tuple(range(n_params, n_params + len(out_names)))

    def _body(*args):
        operands = list(args)
        if partition_name is not None:
            operands.append(bass2jax.partition_id_tensor())
        outs = bass2jax._bass_exec_p.bind(
            *operands,
            out_avals=tuple(out_avals),
            in_names=all_in,
            out_names=tuple(out_names),
            lowering_input_output_aliases=(),
            sim_require_finite=True,
            sim_require_nnan=True,
            nc=nc,
        )
        return tuple(outs)

    mesh = Mesh(np.asarray(devices), ("core",))
    in_specs = (PartitionSpec("core"),) * (n_params + len(out_names))
    out_specs = (PartitionSpec("core"),) * len(out_names)
    sharded = jax.jit(
        jax.shard_map(
            _body, mesh=mesh, in_specs=in_specs, out_specs=out_specs, check_vma=False
        ),
        donate_argnums=donate,
        keep_unused=True,
    )

    def dispatch(in_maps):
        concat_in = [
            np.concatenate([np.asarray(in_maps[c][nm]) for c in range(n)], axis=0)
            for nm in in_names
        ]
        concat_zeros = [
            np.zeros((n * z.shape[0], *z.shape[1:]), z.dtype) for z in zero_outs
        ]
        return sharded(*concat_in, *concat_zeros)

    def unpack(out_arrs):
        return [
            {
                nm: np.asarray(out_arrs[i]).reshape(n, *out_avals[i].shape)[c]
                for i, nm in enumerate(out_names)
            }
            for c in range(n)
        ]

    return dispatch, unpack


def _get_engine():
    if "engine" not in _cache:
        devs = jax.devices()
        ncA = _build_program(SLOT_A, 10)
        ncB = _build_program(SLOT_B, 14)
        dispA, unpackA = _make_runner(ncA, devs[0:4])
        dispB, unpackB = _make_runner(ncB, devs[4:8])
        _cache["engine"] = (dispA, unpackA, dispB, unpackB)
        _cache["ncs"] = (ncA, ncB)
    return _cache["engine"]


def run(query, key, value):
    dispA, unpackA, dispB, unpackB = _get_engine()
    mapsA = [_prep_core_inputs(SLOT_A, b, query, key, value) for b in range(4)]
    mapsB = [_prep_core_inputs(SLOT_B, b, query, key, value) for b in range(4)]
    outA = dispA(mapsA)
    outB = dispB(mapsB)
    resA = unpackA(outA)
    resB = unpackB(outB)

    out = np.zeros((B, S, D), np.float32)
    for b in range(4):
        for slot_c, res in ((SLOT_A, resA[b]), (SLOT_B, resB[b])):
            o = res["o"]
            for m in range(4):
                c = _chunk_index(slot_c, m)
                out[b, 512 * c : 512 * (c + 1), :] = o[512 * m : 512 * (m + 1), :]
    return out


def kernel(query, key, value):
    query = np.ascontiguousarray(np.asarray(query, dtype=np.float32))
    key = np.ascontiguousarray(np.asarray(key, dtype=np.float32))
    value = np.ascontiguousarray(np.asarray(value, dtype=np.float32))
    return run(query, key, value)


# revision 52
# speedup vs baseline: 1.1171x; 1.1171x over previous
"""Causal attention (B=4, S=4096, D=64, fp32) on 8 Trainium2 NeuronCores.

Sharding: two SPMD programs dispatched concurrently on disjoint device
sets. Within a batch, the 4096 q rows form 8 chunks of 512 columns;
chunk c needs k-tiles 0..4c+3 (causal). Program A (cores 0-3, one batch
each) takes chunks {2,0,5,7} (k-tile counts {12,4,24,32}); program B
(cores 4-7) takes chunks {1,3,4,6} (counts {8,16,20,28}). Both sum to
72 tile-passes per core - perfectly balanced. A PE warmup block keeps
the HAM clock-gate from running the stream at 1.2 GHz.

Layout: scores are computed transposed, S^T[k,q] = K Q^T, with the
contraction dim d on SBUF partitions, so softmax normalization can be
deferred (a ones-column appended to V accumulates the row sums during
the P^T V matmul) and P^T feeds the PV matmul with no transposes. The
last up-to-8 k-tiles of each chunk come from per-chunk "slab" inputs;
the final 4 are the diagonal tiles, whose triangles are zeroed by a
DVE multiply with a host-supplied triangular mask at fixed program
positions. All matmul operands are fp16 (PE runs 1 cycle/row);
accumulation stays fp32 in PSUM.
"""

import numpy as np

import jax
import concourse.bass as bass  # noqa: F401
import concourse.mybir as mybir
from concourse import bacc
from concourse import bass2jax
from concourse.tile import TileContext

B, S, D = 4, 4096, 64
NCORES = 8
SLOT_A = (12, 4, 24, 32)  # program A: chunks {2,0,5,7} of a batch (72 tiles)
SLOT_B = (8, 16, 20, 28)  # program B: chunks {1,3,4,6} (72 tiles)
F32 = mybir.dt.float32
F16 = mybir.dt.float16

_cache = {}


def _chunk_index(slot_c, m):
    # chunk whose causal need equals slot_c[m]
    return slot_c[m] // 4 - 1


def _build_program(slot_c, warmup_n):
    n_shared = [max(c - 8, 0) for c in slot_c]
    n_slab = [min(c, 8) for c in slot_c]
    max_shared = max(n_shared)

    nc = bacc.Bacc("TRN2", target_bir_lowering=False, debug=False)
    qt_d = nc.declare_dram_parameter("qt", [65, 2048], F16, isOutput=False)
    ktm_d = nc.declare_dram_parameter(
        "ktm", [64, 128 * max_shared], F16, isOutput=False
    )
    kts_d = nc.declare_dram_parameter("kts", [65, 4096], F16, isOutput=False)
    vm_d = nc.declare_dram_parameter(
        "vm", [128, 65 * max_shared], F16, isOutput=False
    )
    vs_d = nc.declare_dram_parameter("vs", [128, 2080], F16, isOutput=False)
    id_d = nc.declare_dram_parameter("id65", [65, 65], F32, isOutput=False)
    mk_d = nc.declare_dram_parameter("mask", [128, 512], F16, isOutput=False)
    o_d = nc.declare_dram_parameter("o", [2048, 64], F32, isOutput=True)
    EXP = mybir.ActivationFunctionType.Exp

    with TileContext(nc) as tc:
        with (
            tc.tile_pool(name="cons", bufs=1) as cons,
            tc.tile_pool(name="data", bufs=1) as data,
            tc.tile_pool(name="pp", bufs=4) as pp,
            tc.tile_pool(name="ep", bufs=2) as ep,
            tc.tile_pool(name="ps_sc", bufs=3, space="PSUM") as ps_sc,
            tc.tile_pool(name="ps_acc", bufs=1, space="PSUM") as ps_acc,
            tc.tile_pool(name="ps_t", bufs=1, space="PSUM") as ps_t,
        ):
            warm = cons.tile([128, 512], F16)
            nc.vector.memset(warm[:], 0.0)
            for w in range(warmup_n):
                wp = ps_sc.tile([128, 1024], F32, tag="sc")
                nc.tensor.matmul(
                    wp[:, 0:512], warm[:, 0:128], warm[:], start=True, stop=True
                )

            qt = data.tile([65, 2048], F16)
            kts = data.tile([65, 4096], F16)
            vs = data.tile([128, 2080], F16)
            ktm = data.tile([64, 128 * max_shared], F16)
            vm = data.tile([128, 65 * max_shared], F16)

            def dma_slot(m):
                nc.sync.dma_start(
                    out=kts[:, 1024 * m : 1024 * m + 128 * n_slab[m]],
                    in_=kts_d[:, 1024 * m : 1024 * m + 128 * n_slab[m]],
                )
                nc.sync.dma_start(
                    out=vs[:, 520 * m : 520 * m + 65 * n_slab[m]],
                    in_=vs_d[:, 520 * m : 520 * m + 65 * n_slab[m]],
                )

            def dma_main(lo, hi):  # shared k-tiles [lo, hi)
                if hi <= lo:
                    return
                nc.sync.dma_start(
                    out=ktm[:, 128 * lo : 128 * hi], in_=ktm_d[:, 128 * lo : 128 * hi]
                )
                nc.sync.dma_start(
                    out=vm[:, 65 * lo : 65 * hi], in_=vm_d[:, 65 * lo : 65 * hi]
                )

            # ordered by first use so no chunk ever waits on its k data;
            # after each slot's slabs, prefetch one 6-tile piece toward the
            # largest future shared-tile need
            nc.sync.dma_start(out=qt[:], in_=qt_d[:])
            ident = cons.tile([65, 65], F32)
            mask = cons.tile([128, 512], F16)
            done = 0
            for m in range(4):
                ns = n_shared[m]
                while done < ns:
                    step = min(6, ns - done)
                    dma_main(done, done + step)
                    done += step
                dma_slot(m)
                if m == 0:
                    nc.sync.dma_start(out=mask[:], in_=mk_d[:])
                    nc.sync.dma_start(out=ident[:], in_=id_d[:])
                future = max(n_shared[m:])
                if done < future:
                    step = min(6, future - done)
                    dma_main(done, done + step)
                    done += step

            pending = []  # (emit_fn, pt, gang, after_fn) across chunks

            def pump(limit):
                while len(pending) > limit:
                    fn, pt_, gang_, after = pending.pop(0)
                    fn(pt_, gang_)
                    if after is not None:
                        after()

            for m in range(4):
                C = slot_c[m]
                ns = n_shared[m]
                # last chunk: host orders the slab diag-tiles first so the
                # chunk tail is mask-free (shorter kernel-exit chain)
                diag_first = m == 3 and ns >= 4
                q_sl = slice(512 * m, 512 * (m + 1))
                acc = ps_acc.tile([65, 512], F32, tag="acc")

                def tile_geom(t, C=C, ns=ns, diag_first=diag_first):
                    # diagonal tile g only needs q-cols [128g, 512)
                    g = (t - ns) if diag_first else (t - (C - 4))
                    if 0 <= g <= 3:
                        off = 128 * g
                    else:
                        off = 0
                    return (g if 0 <= g <= 3 else -1), off, 512 - off

                def emit_pv(pt, gang, C=C, m=m, ns=ns, acc=acc, tile_geom=tile_geom):
                    pcol = 0
                    for t in gang:
                        g, off, w = tile_geom(t)
                        ptile = pt[:, pcol : pcol + w]
                        pcol += w
                        if g >= 0:
                            nc.vector.tensor_mul(ptile, ptile, mask[:, :w])
                        if t < ns:
                            vt = vm[:, 65 * t : 65 * (t + 1)]
                        else:
                            p = t - ns
                            vt = vs[:, 520 * m + 65 * p : 520 * m + 65 * (p + 1)]
                        nc.tensor.matmul(
                            acc[:, off:512],
                            vt,
                            ptile,
                            start=(t == 0),
                            stop=(t == C - 1),
                        )

                def make_epilogue(m=m, acc=acc):
                    last = m == 3

                    def epilogue():
                        osb = ep.tile([65, 512], F32, tag="osb")
                        if last:
                            # split the PSUM->SBUF copy across DVE and ACT
                            nc.vector.tensor_copy(osb[:, 0:256], acc[:, 0:256])
                            nc.scalar.activation(
                                osb[:, 256:512],
                                acc[:, 256:512],
                                mybir.ActivationFunctionType.Copy,
                            )
                        else:
                            nc.vector.tensor_copy(osb[:], acc[:])
                        oo = ep.tile([128, 256], F32, tag="oo")
                        tp = ps_t.tile([128, 260], F32, tag="tp")
                        for j in range(4):
                            nc.tensor.transpose(
                                tp[:, 65 * j : 65 * (j + 1)],
                                osb[:, 128 * j : 128 * (j + 1)],
                                ident[:],
                            )
                        recs = []
                        for j in range(4):
                            rec = ep.tile([128, 1], F32, tag="rec", bufs=4)
                            nc.vector.reciprocal(
                                rec[:], tp[:, 65 * j + 64 : 65 * j + 65]
                            )
                            recs.append(rec)
                        for j in range(4):
                            if last and j % 2:
                                nc.scalar.activation(
                                    oo[:, 64 * j : 64 * (j + 1)],
                                    tp[:, 65 * j : 65 * j + 64],
                                    mybir.ActivationFunctionType.Copy,
                                    scale=recs[j][:],
                                )
                            else:
                                nc.vector.tensor_scalar_mul(
                                    oo[:, 64 * j : 64 * (j + 1)],
                                    tp[:, 65 * j : 65 * j + 64],
                                    recs[j][:],
                                )
                        halves = ((0, 2), (2, 4)) if last else ((0, 4),)
                        for j0, j1 in halves:
                            nc.sync.dma_start(
                                out=o_d[
                                    512 * m + 128 * j0 : 512 * m + 128 * j1, :
                                ].rearrange("(j p) d -> p j d", j=j1 - j0),
                                in_=oo[:, 64 * j0 : 64 * j1].rearrange(
                                    "p (j d) -> p j d", j=j1 - j0
                                ),
                            )

                    return epilogue

                n_gangs = (C + 1) // 2
                for gi, t0 in enumerate(range(0, C, 2)):
                    gang = list(range(t0, min(t0 + 2, C)))
                    sc = ps_sc.tile([128, 1024], F32, tag="sc")
                    pcol = 0
                    for t in gang:
                        g, off, w = tile_geom(t)
                        if t < ns:
                            lhsT = ktm[:, 128 * t : 128 * (t + 1)]
                            rhs = qt[0:64, q_sl]
                        else:
                            p = t - ns
                            lhsT = kts[
                                :, 1024 * m + 128 * p : 1024 * m + 128 * (p + 1)
                            ]
                            rhs = qt[0:65, q_sl]
                        if off:
                            rhs = rhs[:, off:512]
                        nc.tensor.matmul(
                            sc[:, pcol : pcol + w], lhsT, rhs, start=True, stop=True
                        )
                        pcol += w
                    pt = pp.tile([128, 1024], F16, tag="pt")
                    nc.scalar.activation(
                        pt[:, :pcol], sc[:, :pcol], EXP, scale=0.125
                    )
                    after = make_epilogue() if gi == n_gangs - 1 else None
                    pending.append((emit_pv, pt, gang, after))
                    pump(2)
            pump(0)

    nc.compile()
    return nc


def _prep_core_inputs(slot_c, b, query, key, value):
    n_shared = [max(c - 8, 0) for c in slot_c]
    n_slab = [min(c, 8) for c in slot_c]
    max_shared = max(n_shared)

    qt = np.zeros((65, 2048), np.float16)
    qt[64, :] = 1.0
    kts = np.zeros((65, 4096), np.float16)
    vs = np.zeros((128, 2080), np.float16)
    ktm = np.ascontiguousarray(key[b, : 128 * max_shared, :].T.astype(np.float16))
    vaug = np.ones((S, 65), np.float16)
    vaug[:, :64] = value[b]
    vm = np.ascontiguousarray(
        vaug[: 128 * max_shared]
        .reshape(max_shared, 128, 65)
        .transpose(1, 0, 2)
        .reshape(128, 65 * max_shared)
    )
    for m in range(4):
        c = _chunk_index(slot_c, m)
        n = slot_c[m]
        diag_first = m == 3 and n_shared[m] >= 4
        qt[:64, 512 * m : 512 * (m + 1)] = query[b, 512 * c : 512 * (c + 1), :].T
        for p in range(n_slab[m]):
            if diag_first:
                t = (n - 4 + p) if p < 4 else (n - 8 + (p - 4))
            else:
                t = n_shared[m] + p
            col = slice(1024 * m + 128 * p, 1024 * m + 128 * (p + 1))
            vcol = slice(520 * m + 65 * p, 520 * m + 65 * (p + 1))
            kts[:64, col] = key[b, 128 * t : 128 * (t + 1), :].T
            vs[:, vcol] = vaug[128 * t : 128 * (t + 1), :]
    id65 = np.eye(65, dtype=np.float32)
    mask = np.triu(np.ones((128, 512), dtype=np.float16))
    return {"qt": qt, "ktm": ktm, "kts": kts, "vm": vm, "vs": vs,
            "id65": id65, "mask": mask}


def _make_runner(nc, devices):
    """Vendored multi-core run_bass_via_pjrt with an explicit device set,
    split into an async dispatch and a blocking unpack."""
    from jax.sharding import Mesh, PartitionSpec

    bass2jax.install_neuronx_cc_hook()
    n = len(devices)
    partition_name = nc.partition_id_tensor.name if nc.partition_id_tensor else None
    in_names, out_names, out_avals, zero_outs = [], [], [], []
    for alloc in nc.m.functions[0].allocations:
        if not isinstance(alloc, mybir.MemoryLocationSet):
            continue
        name = alloc.memorylocations[0].name
        if alloc.kind == "ExternalInput":
            if name != partition_name:
                in_names.append(name)
        elif alloc.kind == "ExternalOutput":
            out_names.append(name)
            shape = tuple(alloc.tensor_shape)
            dtype = mybir.dt.np(alloc.dtype)
            out_avals.append(jax.core.ShapedArray(shape, dtype))
            zero_outs.append(np.zeros(shape, dtype))
    n_params = len(in_names)
    all_in = list(in_names) + list(out_names)
    if partition_name is not None:
        all_in.append(partition_name)
    all_in = tuple(all_in)
    donate = tuple(range(n_params, n_params + len(out_names)))

    def _body(*args):
        operands = list(args)
        if partition_name is not None:
            operands.append(bass2jax.partition_id_tensor())
        outs = bass2jax._bass_exec_p.bind(
            *operands,
            out_avals=tuple(out_avals),
            in_names=all_in,
            out_names=tuple(out_names),
            lowering_input_output_aliases=(),
            sim_require_finite=True,
            sim_require_nnan=True,
            nc=nc,
        )
        return tuple(outs)

    mesh = Mesh(np.asarray(devices), ("core",))
    in_specs = (PartitionSpec("core"),) * (n_params + len(out_names))
    out_specs = (PartitionSpec("core"),) * len(out_names)
    sharded = jax.jit(
        jax.shard_map(
            _body, mesh=mesh, in_specs=in_specs, out_specs=out_specs, check_vma=False
        ),
        donate_argnums=donate,
        keep_unused=True,
    )

    def dispatch(in_maps):
        concat_in = [
            np.concatenate([np.asarray(in_maps[c][nm]) for c in range(n)], axis=0)
            for nm in in_names
        ]
        concat_zeros = [
            np.zeros((n * z.shape[0], *z.shape[1:]), z.dtype) for z in zero_outs
        ]
        return sharded(*concat_in, *concat_zeros)

    def unpack(out_arrs):
        return [
            {
                nm: np.asarray(out_arrs[i]).reshape(n, *out_avals[i].shape)[c]
                for i, nm in enumerate(out_names)
            }
            for c in range(n)
        ]

    return dispatch, unpack


def _get_engine():
    if "engine" not in _cache:
        devs = jax.devices()
        ncA = _build_program(SLOT_A, 12)
        ncB = _build_program(SLOT_B, 16)
        dispA, unpackA = _make_runner(ncA, devs[0:4])
        dispB, unpackB = _make_runner(ncB, devs[4:8])
        _cache["engine"] = (dispA, unpackA, dispB, unpackB)
        _cache["ncs"] = (ncA, ncB)
    return _cache["engine"]


def run(query, key, value):
    dispA, unpackA, dispB, unpackB = _get_engine()
    mapsA = [_prep_core_inputs(SLOT_A, b, query, key, value) for b in range(4)]
    mapsB = [_prep_core_inputs(SLOT_B, b, query, key, value) for b in range(4)]
    outA = dispA(mapsA)
    outB = dispB(mapsB)
    resA = unpackA(outA)
    resB = unpackB(outB)

    out = np.zeros((B, S, D), np.float32)
    for b in range(4):
        for slot_c, res in ((SLOT_A, resA[b]), (SLOT_B, resB[b])):
            o = res["o"]
            for m in range(4):
                c = _chunk_index(slot_c, m)
                out[b, 512 * c : 512 * (c + 1), :] = o[512 * m : 512 * (m + 1), :]
    return out


def kernel(query, key, value):
    query = np.ascontiguousarray(np.asarray(query, dtype=np.float32))
    key = np.ascontiguousarray(np.asarray(key, dtype=np.float32))
    value = np.ascontiguousarray(np.asarray(value, dtype=np.float32))
    return run(query, key, value)
